# revision 1
# baseline (speedup 1.0000x reference)
"""Trainium2 Bass kernel for nn_PointWiseMLP (ball query + gather + MLP + pool).

Self-contained: kernel(**inputs) shards across 8 NeuronCores (data-parallel
over batch x query-range), runs the Bass/Tile kernel via run_bass_kernel_spmd,
and gathers the full [2, 128, 8192] output.
"""
import sys
for _p in ("/opt/trn_rl_repo", "/root/.axon_site/_ro/trn_rl_repo"):
    if _p not in sys.path:
        sys.path.append(_p)


import os
import numpy as np
from contextlib import ExitStack

import concourse.bass as bass
import concourse.tile as tile
from concourse import mybir
from concourse._compat import with_exitstack

F32 = mybir.dt.float32
BF16 = mybir.dt.bfloat16
I16 = mybir.dt.int16

RADIUS = 0.1
NSAMPLE = 32
EPS = 1e-5
N2 = 8192
NQ = 2048          # queries per core
NQT = 16           # query tiles per core
BIG = 1000.0
R2 = float(np.float32(0.01))  # threshold as f32

ALU = mybir.AluOpType
ACTF = mybir.ActivationFunctionType


# --------------------------------------------------------------------------
# host-side preparation
# --------------------------------------------------------------------------

def _split_hilo(x):
    """10-bit split: x = hi + lo with hi on 2^-10 grid (exact in f32)."""
    x = x.astype(np.float32)
    hi = np.floor(x.astype(np.float64) * 1024.0) / 1024.0
    hi = hi.astype(np.float32)
    lo = (x - hi).astype(np.float32)
    return hi, lo


def host_prep(inputs):
    B = 2
    qx = np.asarray(inputs['query_xyz'], np.float32)
    sx = np.asarray(inputs['support_xyz'], np.float32)
    qm = np.asarray(inputs['query_mask'], np.int32)
    sm = np.asarray(inputs['support_mask'], np.int32)
    sf = np.asarray(inputs['support_features'], np.float32)

    W0 = np.asarray(inputs['W0'], np.float64)
    W1 = np.asarray(inputs['W1'], np.float64)
    W2 = np.asarray(inputs['W2'], np.float64)

    def fold(Wl, g, b, rm, rv):
        s = np.asarray(g, np.float64) / np.sqrt(np.asarray(rv, np.float64) + EPS)
        return Wl * s[:, None], np.asarray(b, np.float64) - np.asarray(rm, np.float64) * s

    W0p, t0 = fold(W0, inputs['g0'], inputs['b0'], inputs['rm0'], inputs['rv0'])
    W1p, t1 = fold(W1, inputs['g1'], inputs['b1'], inputs['rm1'], inputs['rv1'])
    W2p, t2 = fold(W2, inputs['g2'], inputs['b2'], inputs['rm2'], inputs['rv2'])

    P0 = W0p[:, 0:3] / RADIUS
    C0 = W0p[:, 3:67]
    D0 = W0p[:, 67:131]

    # gvlhs [67, 64]: G weights in cols 0-31, V weights in cols 32-63
    gvlhs = np.zeros((67, 64), np.float32)
    gvlhs[0:64, 0:32] = D0.T
    gvlhs[64:67, 0:32] = P0.T
    gvlhs[0:64, 32:64] = (C0 - D0).T

    p0rT = np.tile(-P0.T.astype(np.float32), (1, 4))       # [3, 128]
    w1t4 = np.tile(W1p.T.astype(np.float32), (4, 1))       # [128, 32]
    w2t = W2p.T.astype(np.float32)                         # [32, 128]

    t0v4 = np.tile(t0.astype(np.float32), 4).reshape(128, 1)
    t1v = t1.astype(np.float32).reshape(32, 1)
    t2v = t2.astype(np.float32).reshape(128, 1)

    # permutation matmul weights for the wrapped gather index layout:
    # idxw[p, 2r+h] = idxg[32*(p//32) + 16h + p%16, r]
    Mh = np.zeros((2, 128, 128), np.float32)
    for h in range(2):
        for p in range(128):
            Mh[h, 32 * (p // 32) + 16 * h + p % 16, p] = 1.0
    ident = np.eye(128, dtype=np.float32)

    pow8 = np.tile((2.0 ** (np.arange(1024) % 8)).astype(np.float32)[None, :], (128, 1))
    iotac = np.tile((np.arange(512, dtype=np.int16) + 1)[None, :], (128, 1))
    shv = np.tile(np.arange(8, dtype=np.int16)[None, :], (128, 1))
    tpat = np.tile((np.tile(np.arange(16, dtype=np.float32), 34) - 15.0)[None, :],
                   (128, 1))
    onesk1 = np.ones((1, 128), np.float32)

    batch_sup = []
    for b in range(B):
        s = sx[b]
        sh, sl = _split_hilo(s)
        s64, sh64 = s.astype(np.float64), sh.astype(np.float64)
        Ls = (np.sum(s64 * s64, 1) - np.sum(sh64 * sh64, 1)).astype(np.float32)
        rhsd2 = np.zeros((13, N2), np.float32)
        rhsd2[0:3] = sh.T
        rhsd2[3] = 1.0
        rhsd2[4] = np.sum(sh * sh, 1, dtype=np.float64).astype(np.float32)
        rhsd2[5:8] = -2.0 * sh.T
        rhsd2[8:11] = -2.0 * sl.T
        rhsd2[11] = 1.0
        rhsd2[12] = Ls + BIG * (1 - sm[b]).astype(np.float32)
        gvrhs = np.zeros((67, N2), np.float32)
        gvrhs[0:64] = sf[b]
        gvrhs[64:67] = s.T
        batch_sup.append((rhsd2, gvrhs))

    import ml_dtypes
    npdt = {F32: np.float32, BF16: ml_dtypes.bfloat16, I16: np.int16}
    in_maps = []
    for c in range(8):
        b = c // 4
        q0 = (c % 4) * NQ
        q = qx[b, q0:q0 + NQ]
        qmk = qm[b, q0:q0 + NQ].astype(np.float32)
        qh, ql = _split_hilo(q)
        q64, qh64 = q.astype(np.float64), qh.astype(np.float64)
        Lq = (np.sum(q64 * q64, 1) - np.sum(qh64 * qh64, 1)).astype(np.float32)
        lhsq = np.zeros((13, NQ), np.float32)
        lhsq[0:3] = -2.0 * qh.T
        lhsq[3] = np.sum(qh * qh, 1, dtype=np.float64).astype(np.float32)
        lhsq[4] = 1.0
        lhsq[5:8] = ql.T
        lhsq[8:11] = q.T
        lhsq[11] = Lq + BIG * (1 - qmk)
        lhsq[12] = 1.0

        rhsd2, gvrhs = batch_sup[b]
        im = dict(
            lhsq=lhsq, rhsd2=rhsd2, gvrhs=gvrhs, gvlhs=gvlhs,
            qt3=q.T.copy(), p0rT=p0rT,
            t0v4=t0v4, t1v=t1v, t2v=t2v,
            w1t4=w1t4, w2t=w2t,
            mh0=Mh[0], mh1=Mh[1], ident=ident,
            pow8=pow8, iotac=iotac, shv=shv, tpat=tpat,
            qfm=qmk.reshape(NQT, 128).T.copy(),
            onesk1=onesk1,
        )
        for k in im:
            shape, dt = IN_SPECS[k]
            arr = np.ascontiguousarray(im[k]).astype(npdt[dt])
            assert arr.shape == shape, (k, arr.shape, shape)
            im[k] = arr
        in_maps.append(im)
    return in_maps


def host_finish(results):
    out = np.zeros((2, 128, 8192), np.float32)
    for c in range(8):
        b = c // 4
        q0 = (c % 4) * NQ
        out[b, :, q0:q0 + NQ] = results[c]['out']
    return out


IN_SPECS = dict(
    lhsq=((13, NQ), F32), rhsd2=((13, N2), F32),
    gvrhs=((67, N2), BF16), gvlhs=((67, 64), BF16),
    qt3=((3, NQ), BF16), p0rT=((3, 128), BF16),
    t0v4=((128, 1), F32), t1v=((32, 1), F32), t2v=((128, 1), F32),
    w1t4=((128, 32), BF16), w2t=((32, 128), BF16),
    mh0=((128, 128), F32), mh1=((128, 128), F32), ident=((128, 128), F32),
    pow8=((128, 1024), BF16), iotac=((128, 512), I16), shv=((128, 8), I16),
    tpat=((128, 544), F32), qfm=((128, NQT), F32), onesk1=((1, 128), F32),
)


# --------------------------------------------------------------------------
# device kernel
# --------------------------------------------------------------------------

@with_exitstack
def build_kernel(ctx: ExitStack, tc: tile.TileContext, out_ap: bass.AP, ins: dict):
    nc = tc.nc
    ctx.enter_context(nc.allow_low_precision("bf16 mlp + exact small-int sums"))

    consts = ctx.enter_context(tc.tile_pool(name="consts", bufs=1))
    gvp = ctx.enter_context(tc.tile_pool(name="gv", bufs=1))
    selp = ctx.enter_context(tc.tile_pool(name="sel", bufs=2))
    selp2 = ctx.enter_context(tc.tile_pool(name="sel2", bufs=2))
    smallp = ctx.enter_context(tc.tile_pool(name="small", bufs=1))
    mlpp = ctx.enter_context(tc.tile_pool(name="mlp", bufs=2))
    mlpp1 = ctx.enter_context(tc.tile_pool(name="mlp1", bufs=1))
    outp = ctx.enter_context(tc.tile_pool(name="outb", bufs=1))
    ps_d2 = ctx.enter_context(tc.tile_pool(name="psd2", bufs=2, space="PSUM"))
    ps_l2 = ctx.enter_context(tc.tile_pool(name="psl2", bufs=1, space="PSUM"))
    ps_l3 = ctx.enter_context(tc.tile_pool(name="psl3", bufs=1, space="PSUM"))

    PROLOGUE_ONLY = ("gvrhs", "gvlhs", "qt3", "p0rT")
    ct = {}
    for name, (shape, dt) in IN_SPECS.items():
        if name in PROLOGUE_ONLY:
            continue
        t = consts.tile(list(shape), dt, tag=f"c_{name}")
        nc.sync.dma_start(out=t[:], in_=ins[name])
        ct[name] = t

    c33 = consts.tile([128, 544], F32, tag="c33")
    nc.vector.memset(c33[:], 33.0)
    ones34 = consts.tile([128, 34], I16, tag="ones34")
    nc.vector.memset(ones34[:], 1)

    # persistent per-core state
    gv4 = gvp.tile([128, 2 * N2], BF16, tag="gv4")     # [128, 8192, 2] (G,V) pairs
    qdB = gvp.tile([128, 512], F32, tag="qdB")         # qdelta, unit-major cols
    idxall = gvp.tile([128, 512], F32, tag="idxall")   # final idx per qtile (f32)
    ceffall = gvp.tile([128, NQT], F32, tag="ceffall")
    outbuf = outp.tile([128, NQ], F32, tag="outbuf")

    # ---- prologue ----
    with tc.tile_pool(name="prolog", bufs=1) as prop:
        pt = {}
        for name in PROLOGUE_ONLY:
            shape, dt = IN_SPECS[name]
            t = prop.tile(list(shape), dt, tag=f"p_{name}")
            nc.sync.dma_start(out=t[:], in_=ins[name])
            pt[name] = t
        gvtmp = prop.tile([64, N2], BF16, tag="gvtmp")
        for n in range(16):
            pgv = ps_l2.tile([64, 512], F32, tag="ps_a")
            nc.tensor.matmul(pgv[:], pt['gvlhs'][:], pt['gvrhs'][:, bass.ts(n, 512)],
                             start=True, stop=True)
            nc.scalar.activation(gvtmp[:, bass.ts(n, 512)], pgv[:], ACTF.Copy)
        # interleave G (rows 0-31) and V (rows 32-63) -> [32, j, 2]; replicate x4
        for uu in range(2):
            nc.sync.dma_start(
                out=gv4[0:32, :].rearrange("p (j u) -> p u j", u=2)[:, uu, :],
                in_=gvtmp[32 * uu:32 * uu + 32, :])
        nc.sync.dma_start(out=gv4[32:64, :], in_=gv4[0:32, :])
        nc.sync.dma_start(out=gv4[64:128, :], in_=gv4[0:64, :])

        # qdelta: qdB[p, i*32 + q] = t0[p%32] - (P0/R)@q(i*128+32*(p//32)+q)
        psQ = ps_l2.tile([128, 512], F32, tag="ps_a")
        for uq in range(4):
            rhs = pt['qt3'][:].rearrange("c (i uu q) -> c uu i q", uu=4, q=32)[:, uq]
            nc.tensor.matmul(psQ[32 * uq:32 * uq + 32, :],
                             pt['p0rT'][:, 32 * uq:32 * uq + 32],
                             rhs, start=True, stop=True,
                             tile_position=(0, 32 * uq))
        nc.vector.tensor_scalar(qdB[:], psQ[:], ct['t0v4'][:], None, ALU.add)

    # ==== interleaved phase blocks: selection then gather+MLP per block ====
    BLK = NQT
    for blk in range(NQT // BLK):
        for i in range(blk * BLK, (blk + 1) * BLK):
            if os.environ.get("SKIP_A"):
                continue
            w8 = selp.tile([128, 1024], BF16, tag="w8")
            for jc in range(8):
                pd2 = ps_d2.tile([128, 1024], F32, tag="ps_d2")
                for n in range(2):
                    nc.tensor.matmul(pd2[:, bass.ts(n, 512)],
                                     ct['lhsq'][:, bass.ts(i, 128)],
                                     ct['rhsd2'][:, bass.ts(2 * jc + n, 512)],
                                     start=True, stop=True)
                vw8c = selp.tile([128, 1024], BF16, tag="vw8c")
                nc.vector.scalar_tensor_tensor(
                    vw8c[:], pd2[:], R2, ct['pow8'][:],
                    op0=ALU.is_lt, op1=ALU.mult)
                nc.vector.tensor_reduce(
                    w8[:, bass.ts(jc, 128)],
                    vw8c[:].rearrange("p (w t) -> p w t", t=8), mybir.AxisListType.X,
                    ALU.add)

            w8v = w8[:].rearrange("p (c two) -> p c two", two=2)
            w8e = selp2.tile([128, 512], I16, tag="w8e")
            w8o = selp2.tile([128, 512], I16, tag="w8o")
            nc.scalar.activation(w8e[:], w8v[:, :, 0], ACTF.Copy)
            nc.scalar.activation(w8o[:], w8v[:, :, 1], ACTF.Copy)

            s16 = smallp.tile([128, 512], F32, tag="s16")
            nc.vector.tensor_tensor(s16[:], w8v[:, :, 0], w8v[:, :, 1], ALU.add)
            nz = smallp.tile([128, 512], F32, tag="nz")
            nc.scalar.activation(nz[:], s16[:], ACTF.Sign)
            crank = smallp.tile([128, 512], F32, tag="crank")
            nc.vector.tensor_tensor_scan(crank[:], nz[:], c33[:, 0:512], 0.0,
                                         ALU.add, ALU.min)
            u = smallp.tile([128, 512], F32, tag="u")
            nc.vector.tensor_tensor(u[:], crank[:], nz[:], ALU.mult)
            v = smallp.tile([128, 512], F32, tag="v")
            nc.vector.scalar_tensor_tensor(v[:], u[:], 32.5, u[:], op0=ALU.is_le,
                                           op1=ALU.mult)
            si16 = selp2.tile([128, 512], I16, tag="si16")
            nc.vector.tensor_scalar(si16[:], v[:], -1.0, None, ALU.add)

            dstID = selp2.tile([128, 34], I16, tag="dstID")
            dstW = selp2.tile([128, 68], I16, tag="dstW")
            nc.gpsimd.local_scatter(dstID[:], ct['iotac'][:], si16[:], 128, 34, 512)
            nc.gpsimd.local_scatter(dstW[:, 0:34], w8e[:], si16[:], 128, 34, 512)
            nc.gpsimd.local_scatter(dstW[:, 34:68], w8o[:], si16[:], 128, 34, 512)

            esel16 = selp2.tile([128, 544], I16, tag="esel16")
            # esel col s*16 + b*8 + t  <-  bit t of dstW col b*34 + s
            evb = esel16[:].rearrange("p (s b t) -> p b s t", b=2, t=8)
            dwv = dstW[:].rearrange("p (b s) -> p b s", b=2)
            ones68 = ones34[:].unsqueeze(1).broadcast_to((128, 2, 34))
            for t in range(8):
                nc.vector.scalar_tensor_tensor(evb[:, :, :, t], dwv,
                                               ct['shv'][:, t:t + 1], ones68,
                                               op0=ALU.logical_shift_right,
                                               op1=ALU.bitwise_and)
            esel = smallp.tile([128, 544], F32, tag="esel")
            nc.scalar.activation(esel[:], esel16[:], ACTF.Copy)
            idf = smallp.tile([128, 34], F32, tag="idf")
            nc.scalar.activation(idf[:], dstID[:], ACTF.Copy)
            cjp1 = selp2.tile([128, 544], I16, tag="cjp1")
            nc.vector.scalar_tensor_tensor(
                cjp1[:].rearrange("p (s t) -> p s t", t=16),
                idf[:].unsqueeze(2).broadcast_to((128, 34, 16)), 16.0,
                ct['tpat'][:].rearrange("p (s t) -> p s t", t=16),
                op0=ALU.mult, op1=ALU.add)

            crank2 = smallp.tile([128, 544], F32, tag="crank2")
            nc.vector.tensor_tensor_scan(crank2[:], esel[:], c33[:], 0.0, ALU.add,
                                         ALU.min)
            u2 = smallp.tile([128, 544], F32, tag="u2")
            nc.vector.tensor_tensor(u2[:], crank2[:], esel[:], ALU.mult)
            v2 = smallp.tile([128, 544], F32, tag="v2")
            nc.vector.scalar_tensor_tensor(v2[:], u2[:], 32.5, u2[:], op0=ALU.is_le,
                                           op1=ALU.mult)
            si2 = selp2.tile([128, 544], I16, tag="si2")
            nc.vector.tensor_scalar(si2[:], v2[:], -1.0, None, ALU.add)
            idxp1 = selp2.tile([128, 34], I16, tag="idxp1")
            nc.gpsimd.local_scatter(idxp1[:], cjp1[:], si2[:], 128, 34, 544)

            # fill + final gather indices (kept in f32 for the phase-B matmul)
            ii = smallp.tile([128, 32], F32, tag="ii")
            nc.scalar.activation(ii[:], idxp1[:, 0:32], ACTF.Copy)
            flp1 = smallp.tile([128, 1], F32, tag="flp1")
            nc.vector.tensor_scalar(flp1[:], ii[:, 0:1], 1.0, None, ALU.max)
            m = smallp.tile([128, 32], F32, tag="m")
            nc.vector.tensor_scalar(m[:], ii[:], 0.0, None, ALU.is_gt)
            bb = smallp.tile([128, 32], F32, tag="bb")
            nc.vector.tensor_tensor(bb[:], ii[:], m[:], ALU.mult)
            aa = smallp.tile([128, 32], F32, tag="aa")
            nc.vector.tensor_scalar(aa[:], m[:], flp1[:], None, ALU.mult)
            cc = smallp.tile([128, 32], F32, tag="cc")
            nc.vector.tensor_tensor(cc[:], bb[:], aa[:], ALU.subtract)
            nc.vector.tensor_scalar(idxall[:, bass.ts(i, 32)], cc[:], flp1[:], -1.0,
                                    ALU.add, ALU.add)

            # effective count (with query-mask fallback to 32)
            cnt0 = smallp.tile([128, 1], F32, tag="cnt0")
            nc.vector.tensor_scalar(cnt0[:], crank2[:, 543:544], 32.0, None, ALU.min)
            qfc = smallp.tile([128, 1], F32, tag="qfc")
            nc.vector.tensor_scalar(qfc[:], ct['qfm'][:, i:i + 1], -32.0, 32.0,
                                    ALU.mult, ALU.add)
            nc.vector.tensor_tensor(ceffall[:, i:i + 1], cnt0[:], qfc[:], ALU.max)

        for i in range(blk * BLK, (blk + 1) * BLK):
            if os.environ.get("SKIP_B"):
                continue
            # wrapped gather index lists via permutation matmuls
            psW = ps_d2.tile([128, 64], F32, tag="ps_d2")
            nc.tensor.matmul(psW[:, 0:32], ct['mh0'][:], idxall[:, bass.ts(i, 32)],
                             start=True, stop=True)
            nc.tensor.matmul(psW[:, 32:64], ct['mh1'][:], idxall[:, bass.ts(i, 32)],
                             start=True, stop=True)
            idxw = smallp.tile([128, 64], I16, tag="idxw")
            nc.scalar.activation(idxw[:, 0::2], psW[:, 0:32], ACTF.Copy)
            nc.scalar.activation(idxw[:, 1::2], psW[:, 32:64], ACTF.Copy)

            gout = mlpp.tile([128, 2048], BF16, tag="gout")
            nc.gpsimd.ap_gather(gout[:].rearrange("p (k u) -> p k u", u=2),
                                gv4[:].rearrange("p (j u) -> p j u", u=2),
                                idxw[:], 128, N2, 2, 1024)
            gv_g = gout[:].rearrange("p (r q u) -> p r q u", r=32, u=2)[:, :, :, 0]
            gv_v0 = gout[:].rearrange("p (k u) -> p k u", u=2)[:, 0:32, 1]

            # d = V(center) + qdelta; h1 = relu(G + d)
            dd = mlpp.tile([128, 32], F32, tag="dd")
            nc.vector.tensor_tensor(dd[:], gv_v0, qdB[:, bass.ts(i, 32)], ALU.add)
            h1t = mlpp1.tile([128, 1024], F32, tag="h1t")
            nc.vector.tensor_tensor(
                h1t[:].rearrange("p (r q) -> p r q", q=32), gv_g,
                dd[:].unsqueeze(1).broadcast_to((128, 32, 32)), ALU.add)
            h1 = mlpp.tile([128, 1024], BF16, tag="h1")
            nc.scalar.activation(h1[:], h1t[:], ACTF.Relu)

            # layer 2: per unit uu (K=32 at partition 32*uu)
            h2 = mlpp1.tile([32, 4096], BF16, tag="h2")
            for uu in range(4):
                psL2 = ps_l2.tile([32, 1024], F32, tag="ps_a")
                for n in range(2):
                    nc.tensor.matmul(
                        psL2[:, bass.ts(n, 512)],
                        ct['w1t4'][32 * uu:32 * uu + 32, :],
                        h1[32 * uu:32 * uu + 32, bass.ts(n, 512)],
                        start=True, stop=True,
                        tile_position=(32 * uu, 0))
                nc.scalar.activation(h2[:, bass.ts(uu, 1024)], psL2[:], ACTF.Relu,
                                     bias=ct['t1v'][:])

            # layer 3
            h3 = mlpp1.tile([128, 4096], BF16, tag="h3")
            for n2 in range(4):
                psL3 = ps_l3.tile([128, 1024], F32, tag="ps_b3")
                for n in range(2):
                    nc.tensor.matmul(psL3[:, bass.ts(n, 512)], ct['w2t'][:],
                                     h2[:, bass.ts(2 * n2 + n, 512)],
                                     start=True, stop=True)
                nc.scalar.activation(h3[:, bass.ts(n2, 1024)], psL3[:], ACTF.Relu,
                                     bias=ct['t2v'][:])

            # pooling
            S = smallp.tile([128, 128], F32, tag="S")
            h30 = smallp.tile([128, 128], F32, tag="h30")
            h3v = h3[:].rearrange("p (a r q) -> p a r q", a=4, r=32)
            for a in range(4):
                nc.vector.tensor_reduce(
                    S[:, bass.ts(a, 32)], h3v[:, a, :, :].transpose([0, 2, 1]),
                    mybir.AxisListType.X, ALU.add)
                nc.scalar.activation(h30[:, bass.ts(a, 32)], h3v[:, a, 0, :], ACTF.Copy)

            # beta/gamma rows via PE transpose + broadcast
            ceff = ceffall[:, i:i + 1]
            beta = smallp.tile([128, 1], F32, tag="beta")
            nc.vector.reciprocal(beta[:], ceff)
            gm0 = smallp.tile([128, 1], F32, tag="gm0")
            nc.vector.tensor_scalar(gm0[:], ceff, -1.0, 32.0, ALU.mult, ALU.add)
            gamma = smallp.tile([128, 1], F32, tag="gamma")
            nc.vector.tensor_tensor(gamma[:], gm0[:], beta[:], ALU.mult)
            psBG = ps_d2.tile([1, 256], F32, tag="ps_d2")
            nc.tensor.matmul(psBG[:, 0:128], beta[:], ct['ident'][:],
                             start=True, stop=True)
            nc.tensor.matmul(psBG[:, 128:256], gamma[:], ct['ident'][:],
                             start=True, stop=True)
            bgrow = smallp.tile([1, 256], F32, tag="bgrow")
            nc.vector.tensor_copy(bgrow[:], psBG[:])
            psB = ps_d2.tile([128, 256], F32, tag="ps_d2")
            nc.tensor.matmul(psB[:], ct['onesk1'][:], bgrow[:], start=True, stop=True)

            e1 = smallp.tile([128, 128], F32, tag="e1")
            nc.vector.tensor_tensor(e1[:], S[:], psB[:, 0:128], ALU.mult)
            e2 = smallp.tile([128, 128], F32, tag="e2")
            nc.vector.tensor_tensor(e2[:], h30[:], psB[:, 128:256], ALU.mult)
            nc.vector.tensor_tensor(outbuf[:, bass.ts(i, 128)], e1[:], e2[:],
                                    ALU.subtract)

    nc.sync.dma_start(out=out_ap, in_=outbuf[:])


# ==========================================================================
# harness entry point: kernel(**inputs) -> full output [2, 128, 8192]
# ==========================================================================

_CACHE = {}


def _build_nc():
    import concourse.bacc as bacc
    import concourse.tile as tile_mod
    nc = bacc.Bacc("TRN2", target_bir_lowering=False, debug=False, num_devices=8)
    in_tiles = {}
    for name, (shape, dt) in IN_SPECS.items():
        in_tiles[name] = nc.dram_tensor(
            name, list(shape), dt, kind="ExternalInput").ap()
    out_tile = nc.dram_tensor("out", (128, NQ), F32, kind="ExternalOutput").ap()
    with tile_mod.TileContext(nc) as t:
        build_kernel(t, out_tile, in_tiles)
    nc.compile()
    return nc


def kernel(**inputs):
    from concourse.bass_utils import run_bass_kernel_spmd
    in_maps = host_prep(inputs)
    if "nc" not in _CACHE:
        _CACHE["nc"] = _build_nc()
    res = run_bass_kernel_spmd(_CACHE["nc"], in_maps, list(range(8)))
    return host_finish(res.results)

